# revision 18
# baseline (speedup 1.0000x reference)
"""Trainium2 Bass kernel for nn_BurgersSolver_75333726371954.

Burgers' equation explicit solver: interpolate u0 [64,512] to a 513-point
grid, run 5000 sequential periodic-stencil steps on [64,512], snapshot every
50th step at every 2nd spatial point -> [64,257,101].

Strategy (pure data parallel, batch sharded 8 rows/core across 8 cores):
  * Scaled state w = C1*u so the update is
        w' = LIN*w + w_l*(C2+w) + w_r*(C2-w),   LIN = 1-2*C2
  * A custom DVE op (BURGERS_STEP2_ANT) computes one full time step per
    PASS over the stream, using two depth-1 temporal feedback taps
    (CURR_ALU_OUT z^-1 of the input stream for the left term, and z^-1 of
    an intermediate accumulator for combining the right term).  One input
    stream + one output stream; out base = in base - 1.
  * MULTI-PASS instructions: both APs get a stride-0 middle dim [K], so
    ONE instruction = K time steps (the ~75ns/instruction sequencer
    overhead and the ~12ns/pass row-restart are the only overheads beyond
    the ~1.05ns/element 1x DVE rate).  K <= 13 (hw row-count limit) and
    stream width >= ~104 elements (the src prefetcher runs ~103 elements
    ahead of compute; narrower passes read stale data -- PAD widens the
    stream to 110 for margin).
  * Layout [128 partitions = 8 batch x 16 spatial chunks of 32 sites,
    free = 2 dead cols + (H | 32 core | H) + 28 pad].  Ghost depth H=25
    allows 25 steps (K=12+13, two instructions) between halo refreshes;
    the valid region tapers by 1/side/step.
  * Halo refresh via 2 DVE stream_shuffle copies (partition rotation +-1
    within each 16-chunk group).  Everything runs on the vector engine in
    program order -- no PE, no PSUM, no cross-engine semaphores.
  * Snapshots every 50 steps land on block boundaries (50 = 2 x H); a
    custom 1x strided copy into an SBUF accumulator (doubling as the
    ghost-writeback spacer), single DMA out at the end.  Host rescales
    by 1/C1 and assembles the [64,257,101] output.
"""

import numpy as np

# ---- problem constants (hardcoded; must match the reference config) ----
MX = 513
MT = 5001
DX = 1.0 / (MX - 1)
DT = 1.0 / (MT - 1)
C1 = DT / (2.0 * DX)            # 0.0512
C2 = 0.005 * DT / DX ** 2       # 0.262144
LIN = float(1.0 - 2.0 * C2)

NSTEPS = MT - 1                 # 5000
SNAP_EVERY = 50
NSNAP = NSTEPS // SNAP_EVERY + 1  # 101

NCORES = 8
BPC = 8                         # batch rows per core
NCHUNK = 16                     # spatial chunks per batch row
CH = 32                         # chunk width (NCHUNK*CH == 512)
H = 25                          # ghost depth == steps per block
KSPLIT = (12, 13)               # passes per DVE instruction (sum == H; <= 13)
SW = CH + 2 * H                 # live state width (84)
PAD = 28                        # junk pad to keep the stream above the
                                # ~104-element src-prefetch staleness floor
WL = SW + PAD                   # instruction stream width (114)
TB = 2                          # dead leading cols (garbage landing zone)
TW = TB + WL                    # tile free width

_COMPILED = {}

# ---------------------------------------------------------------------------
# Custom DVE op: one Burgers step per pass, single input stream.
#
#   w'[c] = LIN*w[c] + w[c-1]*(C2+w[c]) + w[c+1]*(C2-w[c])
#
# Stream x = in0 (base b).  At position t (cur = x[b+t], xp = z^-1):
#   t_a = C2 + cur; t_b = C2 - xp; Q = xp*t_a; P = cur*t_b
#   A = LIN*cur + Q;  out[t] = z^-1(A) + P     -> center c = b+t-1
# out positions 0,1 are garbage (stale taps) and land on taper columns.
# ---------------------------------------------------------------------------

OP_NAME = "BURGERS_STEP2_ANT"


def _stencil_ref(in0, in1, s0, s1, imm2):
    out = np.zeros_like(in0)
    x = in0
    cur, xl, xr = x[:, 1:-1], x[:, :-2], x[:, 2:]
    out[:, 2:] = s0 * cur + xl * (s1 + cur) + xr * (s1 - cur)
    return out


def _build_uops():
    from concourse.dve_uop import (
        ENABLE, AluInp, AluOp, DelayInp, InpSel, OutPath, OutSel, Trigger,
        UopConfig,
    )

    u = UopConfig()
    u.enable_input(InpSel.SRC_0, 1)    # lane1 -> blk0's PREV_DELAY_0 port
    u.enable_input(InpSel.CONST_0, 2)  # LIN   -> PREV_DELAY_1
    u.enable_input(InpSel.CONST_1, 3)  # C2    -> PREV_DELAY_2
    u.require_inp0 = ENABLE
    u.trigger = (Trigger.SRC_TENSOR_DONE, Trigger.NONE, Trigger.NONE)
    u.next_uop = (0, 0, 0)
    u.enable_output(OutSel.ALU_OUT, OutPath.WR0_LO)
    dp = u.datapath_config
    # slice 0: stream -> flop0 (cur); d0 = z^-1 = xp; constants ride d1, d2
    dp[0].enable_alu(AluOp.BYPASS, AluInp.PREV_DELAY_0)
    dp[0].enable_delay_from_src(DelayInp.CURR_ALU_OUT, 0)    # d0 = xp
    dp[0].pass_through_delay(1, 2)
    # slice 1: t_a = cur + C2 ; keep cur in d3
    dp[1].enable_alu(AluOp.ADD, AluInp.PREV_ALU_OUT, AluInp.PREV_DELAY_2)
    dp[1].enable_delay_from_src(DelayInp.PREV_ALU_OUT, 3)    # d3 = cur
    dp[1].pass_through_delay(0, 1, 2)
    # slice 2: t_b = C2 - xp ; keep t_a in d4
    dp[2].enable_alu(AluOp.SUBTRACT, AluInp.PREV_DELAY_2, AluInp.PREV_DELAY_0)
    dp[2].enable_delay_from_src(DelayInp.PREV_ALU_OUT, 4)    # d4 = t_a
    dp[2].pass_through_delay(0, 1, 3)
    # slice 3: Q = xp * t_a ; keep t_b in d5
    dp[3].enable_alu(AluOp.MULTIPLY, AluInp.PREV_DELAY_0, AluInp.PREV_DELAY_4)
    dp[3].enable_delay_from_src(DelayInp.PREV_ALU_OUT, 5)    # d5 = t_b
    dp[3].pass_through_delay(1, 3)
    # slice 4: P = cur * t_b ; keep Q in d0
    dp[4].enable_alu(AluOp.MULTIPLY, AluInp.PREV_DELAY_3, AluInp.PREV_DELAY_5)
    dp[4].enable_delay_from_src(DelayInp.PREV_ALU_OUT, 0)    # d0 = Q
    dp[4].pass_through_delay(1, 3)
    # slice 5: A_lin = LIN * cur ; keep P in d1
    dp[5].enable_alu(AluOp.MULTIPLY, AluInp.PREV_DELAY_1, AluInp.PREV_DELAY_3)
    dp[5].enable_delay_from_src(DelayInp.PREV_ALU_OUT, 1)    # d1 = P
    dp[5].pass_through_delay(0)
    # slice 6: A = A_lin + Q ; d2 = z^-1(A) from own flop
    dp[6].enable_alu(AluOp.ADD, AluInp.PREV_ALU_OUT, AluInp.PREV_DELAY_0)
    dp[6].enable_delay_from_src(DelayInp.CURR_ALU_OUT, 2)    # d2 = A[t-1]
    dp[6].pass_through_delay(1)
    # slice 7: out = z^-1(A) + P
    dp[7].enable_alu(AluOp.ADD, AluInp.PREV_DELAY_2, AluInp.PREV_DELAY_1)
    u.validate("v3")
    return [u]


COPY_NAME = "COPY1X_ANT"


def _build_copy_uops():
    from concourse.dve_uop import (
        ENABLE, AluInp, AluOp, InpSel, OutPath, OutSel, Trigger, UopConfig,
    )

    u = UopConfig()
    u.enable_input(InpSel.SRC_0, 1)
    u.require_inp0 = ENABLE
    u.trigger = (Trigger.SRC_TENSOR_DONE, Trigger.NONE, Trigger.NONE)
    u.next_uop = (0, 0, 0)
    u.enable_output(OutSel.ALU_OUT, OutPath.WR0_LO)
    dp = u.datapath_config
    dp[0].enable_alu(AluOp.BYPASS, AluInp.PREV_DELAY_0)
    for i in range(1, 8):
        dp[i].enable_alu(AluOp.BYPASS, AluInp.PREV_ALU_OUT)
    u.validate("v3")
    return [u]


class _RawDveOp:
    """Duck-typed DveOp whose compile() returns hand-built uops."""

    def __init__(self, name, uop_builder, reference):
        from concourse.dve_spec import Spec, Src0

        self.name = name
        self.subdim = False
        self.perf_en = {}
        self.spec = Spec(body=Src0, reference=reference)
        self._uop_builder = uop_builder
        self._compiled = {}

    def compile(self, ver):
        if ver not in self._compiled:
            from concourse.dve_ops import get_dve_sub_opcode
            from concourse.dve_uop import DveOpSpec

            self._compiled[ver] = DveOpSpec(
                name=self.name,
                opcode=get_dve_sub_opcode(self.name),
                uops=self._uop_builder(),
                rd1_en=False,
            )
        return self._compiled[ver]


def _register_op(name, uop_builder, reference):
    import concourse.dve_ops as dve_ops

    for op in dve_ops.OPS:
        if op.name == name:
            return op
    op = _RawDveOp(name, uop_builder, reference)
    dve_ops.OPS.append(op)
    dve_ops.CUSTOM_DVE_SPECS[name] = op.spec
    dve_ops._SUB_OPCODE_FOR_NAME[name] = (
        max(dve_ops._SUB_OPCODE_FOR_NAME.values()) + 1
    )
    assert dve_ops._SUB_OPCODE_FOR_NAME[name] < 0x20
    return op


def _register_stencil_op():
    return _register_op(OP_NAME, _build_uops, _stencil_ref)


def _register_copy_op():
    return _register_op(COPY_NAME, _build_copy_uops,
                        lambda in0, in1, s0, s1, imm2: in0)


def _rot_mask(d):
    """out lane l takes input lane (l%16 + d)%16 within its 16-chunk group."""
    return [(l // 16) * 16 + ((l % 16) + d) % 16 for l in range(32)]


# ghost refresh pieces: (dst_lo, dst_hi, src_lo, rotation)  [state cols]
# left ghosts [0,H)  <- chunk c-1 cores [CH, CH+H) = state [2H-... wait:
#   ghost col g in [0,H): site offset g-H -> chunk c-1 col g-H+CH
#   = state col g-H+CH+H = g+CH
# right ghosts [H+CH, SW) <- chunk c+1 cols [0, H) = state cols [H, 2H)
_GHOSTS = [
    (0, H, CH, -1),               # left  [0,25)  <- c-1 state [32,57)
    (H + CH, SW, H, +1),          # right [57,84) <- c+1 state [25,50)
]


def _build():
    import concourse.bass as bass
    import concourse.mybir as mybir
    from concourse.ap import AP

    stencil = _register_stencil_op()
    copy1x = _register_copy_op()

    F32 = mybir.dt.float32

    nc = bass.Bass()
    x_in = nc.dram_tensor("x", [128, TW], F32, kind="ExternalInput")
    y_out = nc.dram_tensor("y", [128, NSNAP * 16], F32, kind="ExternalOutput")

    n_blocks = NSTEPS // H
    assert NSTEPS % H == 0 and sum(KSPLIT) == H and SNAP_EVERY % H == 0

    with (
        nc.semaphore("dma_sem") as dma_sem,
        nc.semaphore("v_sem") as v_sem,
        nc.sbuf_tensor("U", [128, TW], F32) as U,
        nc.sbuf_tensor("SN", [128, NSNAP * 16], F32) as SN,
        nc.sbuf_tensor("ZZ", [128, 1], F32) as ZZ,
        nc.sbuf_tensor("SC", [128, 4], F32) as SC,
    ):
        ub = U[:]
        ps = ub.ap[0][0]
        aps = {k: (AP(ub.tensor, TB, [[ps, 128], [0, k], [1, WL]]),
                   AP(ub.tensor, TB - 1, [[ps, 128], [0, k], [1, WL]]))
               for k in set(KSPLIT)}

        with nc.Block() as block:
            @block.gpsimd
            def _(g):
                g.memset(ZZ[:], 0.0)
                g.memset(SC[:], 0.0)
                g.dma_start(U[:], x_in[:]).then_inc(dma_sem, 16)

            zbc2 = ZZ[:].to_broadcast([128, 2])

            @block.vector
            def _(v):
                v.wait_ge(dma_sem, 16)

                def snapshot(k):
                    # custom 1x copy (a stock 2x copy could outrun writeback)
                    v._custom_dve(copy1x, out=SN[:, k * 16:k * 16 + 16],
                                  in0=U[:, TB + H:TB + H + CH:2])

                snapshot(0)
                snap = 1
                step = 0
                for blk in range(n_blocks):
                    for k in KSPLIT:
                        in0, out = aps[k]
                        v._custom_dve(stencil, out=out, in0=in0,
                                      s0=LIN, s1=C2)
                        step += k
                    if blk < n_blocks - 1:
                        for dlo, dhi, slo, rot in _GHOSTS:
                            w = dhi - dlo
                            v.stream_shuffle(U[:, TB + dlo:TB + dhi],
                                             U[:, TB + slo:TB + slo + w],
                                             _rot_mask(rot))
                    # snapshot doubles as the writeback-margin spacer between
                    # the ghost shuffles and the next step instruction's
                    # prefetch; on non-snapshot blocks use a dummy spacer
                    if step % SNAP_EVERY == 0:
                        snapshot(snap)
                        snap += 1
                    elif blk < n_blocks - 1:
                        v._custom_dve(copy1x, out=SC[:, 0:2], in0=SC[:, 2:4])
                assert snap == NSNAP, snap
                v.tensor_add(SC[:, 0:2], SC[:, 2:4], zbc2).then_inc(v_sem, 1)

            @block.gpsimd
            def _(g):
                g.wait_ge(v_sem, 1)
                g.dma_start(y_out[:], SN[:]).then_inc(dma_sem, 16)
                g.wait_ge(dma_sem, 32)

    mybir.codegen_inst_isa_subclasses(nc)
    return nc


def _interp_init(u0):
    """Replicate the reference's 1D border-padded linear interp, f32."""
    u0 = np.asarray(u0, dtype=np.float32)
    n_in = u0.shape[1]
    X = np.linspace(0.0, 1.0, MX, dtype=np.float32)
    pts = X * np.float32(2.0) - np.float32(1.0)
    idx = (pts + np.float32(1.0)) * np.float32(0.5) * np.float32(n_in - 1)
    idx = np.clip(idx, 0.0, np.float32(n_in - 1))
    i0 = np.floor(idx).astype(np.int32)
    i0 = np.clip(i0, 0, n_in - 2)
    frac = (idx - i0.astype(np.float32)).astype(np.float32)
    u0f = u0[:, i0] * (np.float32(1.0) - frac) + u0[:, i0 + 1] * frac
    return u0f[:, :-1].astype(np.float32)   # [B, 512]


def _in_maps(u0):
    """Per-core input tiles [128, TW]: dead cols + prefilled ghosts + pad."""
    u_init = _interp_init(u0)                       # [64, 512]
    w0 = (np.float32(C1) * u_init).astype(np.float32)
    cc, xx = np.meshgrid(np.arange(NCHUNK), np.arange(TW), indexing="ij")
    src = (cc * CH + xx - TB - H) % 512             # [16, TW]
    maps = []
    for core in range(NCORES):
        wrows = w0[core * BPC:(core + 1) * BPC]     # [8, 512]
        tile = wrows[:, src].astype(np.float32)     # [8, 16, TW]
        maps.append({"x": tile.reshape(128, TW)})
    return maps


def kernel(u0):
    from concourse.bass_utils import run_bass_kernel_spmd

    u0 = np.asarray(u0, dtype=np.float32)
    B = u0.shape[0]
    assert B == NCORES * BPC and u0.shape[1] == 512

    if "nc" not in _COMPILED:
        _COMPILED["nc"] = _build()
    nc = _COMPILED["nc"]

    res = run_bass_kernel_spmd(nc, _in_maps(u0), core_ids=list(range(NCORES)))

    out = np.empty((B, 257, NSNAP), dtype=np.float32)
    inv_c1 = np.float32(1.0 / C1)
    for core in range(NCORES):
        y = res.results[core]["y"]                  # [128, NSNAP*16]
        y = y.reshape(BPC, NCHUNK, NSNAP, 16)       # [b, chunk, t, k]
        u = y * inv_c1
        # spatial index nx = chunk*16 + k  (covers 0..255)
        out[core * BPC:(core + 1) * BPC, 0:256, :] = (
            u.transpose(0, 1, 3, 2).reshape(BPC, 256, NSNAP))
    out[:, 256, :] = out[:, 0, :]
    return out


# revision 19
# speedup vs baseline: 1.0160x; 1.0160x over previous
"""Trainium2 Bass kernel for nn_BurgersSolver_75333726371954.

Burgers' equation explicit solver: interpolate u0 [64,512] to a 513-point
grid, run 5000 sequential periodic-stencil steps on [64,512], snapshot every
50th step at every 2nd spatial point -> [64,257,101].

Strategy (pure data parallel, batch sharded 8 rows/core across 8 cores):
  * Scaled state w = C1*u so the update is
        w' = LIN*w + w_l*(C2+w) + w_r*(C2-w),   LIN = 1-2*C2
  * A custom DVE op (BURGERS_STEP2_ANT) computes one full time step per
    PASS over the stream, using two depth-1 temporal feedback taps
    (CURR_ALU_OUT z^-1 of the input stream for the left term, and z^-1 of
    an intermediate accumulator for combining the right term).  One input
    stream + one output stream; out base = in base - 1.
  * MULTI-PASS instructions: both APs get a stride-0 middle dim [K], so
    ONE instruction = K time steps (the ~75ns/instruction sequencer
    overhead and the ~12ns/pass row-restart are the only overheads beyond
    the ~1.05ns/element 1x DVE rate).  K <= 13 (hw row-count limit) and
    stream width >= ~104 elements (the src prefetcher runs ~103 elements
    ahead of compute; narrower passes read stale data -- PAD widens the
    stream to 110 for margin).
  * Layout [128 partitions = 8 batch x 16 spatial chunks of 32 sites,
    free = 2 dead cols + (H | 32 core | H) + 28 pad].  Ghost depth H=25
    allows 25 steps (K=12+13, two instructions) between halo refreshes;
    the valid region tapers by 1/side/step.
  * Halo refresh via 2 DVE stream_shuffle copies (partition rotation +-1
    within each 16-chunk group).  Everything runs on the vector engine in
    program order -- no PE, no PSUM, no cross-engine semaphores.
  * Snapshots every 50 steps land on block boundaries (50 = 2 x H); a
    custom 1x strided copy into an SBUF accumulator (doubling as the
    ghost-writeback spacer), single DMA out at the end.  Host rescales
    by 1/C1 and assembles the [64,257,101] output.
"""

import numpy as np

# ---- problem constants (hardcoded; must match the reference config) ----
MX = 513
MT = 5001
DX = 1.0 / (MX - 1)
DT = 1.0 / (MT - 1)
C1 = DT / (2.0 * DX)            # 0.0512
C2 = 0.005 * DT / DX ** 2       # 0.262144
LIN = float(1.0 - 2.0 * C2)

NSTEPS = MT - 1                 # 5000
SNAP_EVERY = 50
NSNAP = NSTEPS // SNAP_EVERY + 1  # 101

NCORES = 8
BPC = 8                         # batch rows per core
NCHUNK = 16                     # spatial chunks per batch row
CH = 32                         # chunk width (NCHUNK*CH == 512)
H = 25                          # ghost depth == steps per block
KSPLIT = (12, 13)               # passes per DVE instruction (sum == H; <= 13)
SW = CH + 2 * H                 # live state width (84)
PAD = 26                        # junk pad to keep the stream above the
                                # ~104-element src-prefetch staleness floor
WL = SW + PAD                   # instruction stream width (114)
TB = 2                          # dead leading cols (garbage landing zone)
TW = TB + WL                    # tile free width

_COMPILED = {}

# ---------------------------------------------------------------------------
# Custom DVE op: one Burgers step per pass, single input stream.
#
#   w'[c] = LIN*w[c] + w[c-1]*(C2+w[c]) + w[c+1]*(C2-w[c])
#
# Stream x = in0 (base b).  At position t (cur = x[b+t], xp = z^-1):
#   t_a = C2 + cur; t_b = C2 - xp; Q = xp*t_a; P = cur*t_b
#   A = LIN*cur + Q;  out[t] = z^-1(A) + P     -> center c = b+t-1
# out positions 0,1 are garbage (stale taps) and land on taper columns.
# ---------------------------------------------------------------------------

OP_NAME = "BURGERS_STEP2_ANT"


def _stencil_ref(in0, in1, s0, s1, imm2):
    out = np.zeros_like(in0)
    x = in0
    cur, xl, xr = x[:, 1:-1], x[:, :-2], x[:, 2:]
    out[:, 2:] = s0 * cur + xl * (s1 + cur) + xr * (s1 - cur)
    return out


def _build_uops():
    from concourse.dve_uop import (
        ENABLE, AluInp, AluOp, DelayInp, InpSel, OutPath, OutSel, Trigger,
        UopConfig,
    )

    u = UopConfig()
    u.enable_input(InpSel.SRC_0, 1)    # lane1 -> blk0's PREV_DELAY_0 port
    u.enable_input(InpSel.CONST_0, 2)  # LIN   -> PREV_DELAY_1
    u.enable_input(InpSel.CONST_1, 3)  # C2    -> PREV_DELAY_2
    u.require_inp0 = ENABLE
    u.trigger = (Trigger.SRC_TENSOR_DONE, Trigger.NONE, Trigger.NONE)
    u.next_uop = (0, 0, 0)
    u.enable_output(OutSel.ALU_OUT, OutPath.WR0_LO)
    dp = u.datapath_config
    # slice 0: stream -> flop0 (cur); d0 = z^-1 = xp; constants ride d1, d2
    dp[0].enable_alu(AluOp.BYPASS, AluInp.PREV_DELAY_0)
    dp[0].enable_delay_from_src(DelayInp.CURR_ALU_OUT, 0)    # d0 = xp
    dp[0].pass_through_delay(1, 2)
    # slice 1: t_a = cur + C2 ; keep cur in d3
    dp[1].enable_alu(AluOp.ADD, AluInp.PREV_ALU_OUT, AluInp.PREV_DELAY_2)
    dp[1].enable_delay_from_src(DelayInp.PREV_ALU_OUT, 3)    # d3 = cur
    dp[1].pass_through_delay(0, 1, 2)
    # slice 2: t_b = C2 - xp ; keep t_a in d4
    dp[2].enable_alu(AluOp.SUBTRACT, AluInp.PREV_DELAY_2, AluInp.PREV_DELAY_0)
    dp[2].enable_delay_from_src(DelayInp.PREV_ALU_OUT, 4)    # d4 = t_a
    dp[2].pass_through_delay(0, 1, 3)
    # slice 3: Q = xp * t_a ; keep t_b in d5
    dp[3].enable_alu(AluOp.MULTIPLY, AluInp.PREV_DELAY_0, AluInp.PREV_DELAY_4)
    dp[3].enable_delay_from_src(DelayInp.PREV_ALU_OUT, 5)    # d5 = t_b
    dp[3].pass_through_delay(1, 3)
    # slice 4: P = cur * t_b ; keep Q in d0
    dp[4].enable_alu(AluOp.MULTIPLY, AluInp.PREV_DELAY_3, AluInp.PREV_DELAY_5)
    dp[4].enable_delay_from_src(DelayInp.PREV_ALU_OUT, 0)    # d0 = Q
    dp[4].pass_through_delay(1, 3)
    # slice 5: A_lin = LIN * cur ; keep P in d1
    dp[5].enable_alu(AluOp.MULTIPLY, AluInp.PREV_DELAY_1, AluInp.PREV_DELAY_3)
    dp[5].enable_delay_from_src(DelayInp.PREV_ALU_OUT, 1)    # d1 = P
    dp[5].pass_through_delay(0)
    # slice 6: A = A_lin + Q ; d2 = z^-1(A) from own flop
    dp[6].enable_alu(AluOp.ADD, AluInp.PREV_ALU_OUT, AluInp.PREV_DELAY_0)
    dp[6].enable_delay_from_src(DelayInp.CURR_ALU_OUT, 2)    # d2 = A[t-1]
    dp[6].pass_through_delay(1)
    # slice 7: out = z^-1(A) + P
    dp[7].enable_alu(AluOp.ADD, AluInp.PREV_DELAY_2, AluInp.PREV_DELAY_1)
    u.validate("v3")
    return [u]


COPY_NAME = "COPY1X_ANT"


def _build_copy_uops():
    from concourse.dve_uop import (
        ENABLE, AluInp, AluOp, InpSel, OutPath, OutSel, Trigger, UopConfig,
    )

    u = UopConfig()
    u.enable_input(InpSel.SRC_0, 1)
    u.require_inp0 = ENABLE
    u.trigger = (Trigger.SRC_TENSOR_DONE, Trigger.NONE, Trigger.NONE)
    u.next_uop = (0, 0, 0)
    u.enable_output(OutSel.ALU_OUT, OutPath.WR0_LO)
    dp = u.datapath_config
    dp[0].enable_alu(AluOp.BYPASS, AluInp.PREV_DELAY_0)
    for i in range(1, 8):
        dp[i].enable_alu(AluOp.BYPASS, AluInp.PREV_ALU_OUT)
    u.validate("v3")
    return [u]


class _RawDveOp:
    """Duck-typed DveOp whose compile() returns hand-built uops."""

    def __init__(self, name, uop_builder, reference):
        from concourse.dve_spec import Spec, Src0

        self.name = name
        self.subdim = False
        self.perf_en = {}
        self.spec = Spec(body=Src0, reference=reference)
        self._uop_builder = uop_builder
        self._compiled = {}

    def compile(self, ver):
        if ver not in self._compiled:
            from concourse.dve_ops import get_dve_sub_opcode
            from concourse.dve_uop import DveOpSpec

            self._compiled[ver] = DveOpSpec(
                name=self.name,
                opcode=get_dve_sub_opcode(self.name),
                uops=self._uop_builder(),
                rd1_en=False,
            )
        return self._compiled[ver]


def _register_op(name, uop_builder, reference):
    import concourse.dve_ops as dve_ops

    for op in dve_ops.OPS:
        if op.name == name:
            return op
    op = _RawDveOp(name, uop_builder, reference)
    dve_ops.OPS.append(op)
    dve_ops.CUSTOM_DVE_SPECS[name] = op.spec
    dve_ops._SUB_OPCODE_FOR_NAME[name] = (
        max(dve_ops._SUB_OPCODE_FOR_NAME.values()) + 1
    )
    assert dve_ops._SUB_OPCODE_FOR_NAME[name] < 0x20
    return op


def _register_stencil_op():
    return _register_op(OP_NAME, _build_uops, _stencil_ref)


def _register_copy_op():
    return _register_op(COPY_NAME, _build_copy_uops,
                        lambda in0, in1, s0, s1, imm2: in0)


def _rot_mask(d):
    """out lane l takes input lane (l%16 + d)%16 within its 16-chunk group."""
    return [(l // 16) * 16 + ((l % 16) + d) % 16 for l in range(32)]


# ghost refresh pieces: (dst_lo, dst_hi, src_lo, rotation)  [state cols]
# left ghosts [0,H)  <- chunk c-1 cores [CH, CH+H) = state [2H-... wait:
#   ghost col g in [0,H): site offset g-H -> chunk c-1 col g-H+CH
#   = state col g-H+CH+H = g+CH
# right ghosts [H+CH, SW) <- chunk c+1 cols [0, H) = state cols [H, 2H)
_GHOSTS = [
    (0, H, CH, -1),               # left  [0,25)  <- c-1 state [32,57)
    (H + CH, SW, H, +1),          # right [57,84) <- c+1 state [25,50)
]


def _build():
    import concourse.bass as bass
    import concourse.mybir as mybir
    from concourse.ap import AP

    stencil = _register_stencil_op()
    copy1x = _register_copy_op()

    F32 = mybir.dt.float32

    nc = bass.Bass()
    x_in = nc.dram_tensor("x", [128, TW], F32, kind="ExternalInput")
    y_out = nc.dram_tensor("y", [128, NSNAP * 16], F32, kind="ExternalOutput")

    n_blocks = NSTEPS // H
    assert NSTEPS % H == 0 and sum(KSPLIT) == H and SNAP_EVERY % H == 0

    with (
        nc.semaphore("dma_sem") as dma_sem,
        nc.semaphore("v_sem") as v_sem,
        nc.sbuf_tensor("U", [128, TW], F32) as U,
        nc.sbuf_tensor("SN", [128, NSNAP * 16], F32) as SN,
        nc.sbuf_tensor("ZZ", [128, 1], F32) as ZZ,
        nc.sbuf_tensor("SC", [128, 4], F32) as SC,
    ):
        ub = U[:]
        ps = ub.ap[0][0]
        aps = {k: (AP(ub.tensor, TB, [[ps, 128], [0, k], [1, WL]]),
                   AP(ub.tensor, TB - 1, [[ps, 128], [0, k], [1, WL]]))
               for k in set(KSPLIT)}

        with nc.Block() as block:
            @block.gpsimd
            def _(g):
                g.memset(ZZ[:], 0.0)
                g.memset(SC[:], 0.0)
                g.dma_start(U[:], x_in[:]).then_inc(dma_sem, 16)

            zbc2 = ZZ[:].to_broadcast([128, 2])

            @block.vector
            def _(v):
                v.wait_ge(dma_sem, 16)

                def snapshot(k):
                    # custom 1x copy (a stock 2x copy could outrun writeback)
                    v._custom_dve(copy1x, out=SN[:, k * 16:k * 16 + 16],
                                  in0=U[:, TB + H:TB + H + CH:2])

                snapshot(0)
                snap = 1
                step = 0
                for blk in range(n_blocks):
                    for k in KSPLIT:
                        in0, out = aps[k]
                        v._custom_dve(stencil, out=out, in0=in0,
                                      s0=LIN, s1=C2)
                        step += k
                    if blk < n_blocks - 1:
                        for dlo, dhi, slo, rot in _GHOSTS:
                            w = dhi - dlo
                            v.stream_shuffle(U[:, TB + dlo:TB + dhi],
                                             U[:, TB + slo:TB + slo + w],
                                             _rot_mask(rot))
                    # snapshot doubles as the writeback-margin spacer between
                    # the ghost shuffles and the next step instruction's
                    # prefetch; on non-snapshot blocks use a dummy spacer
                    if step % SNAP_EVERY == 0:
                        snapshot(snap)
                        snap += 1
                    elif blk < n_blocks - 1:
                        v._custom_dve(copy1x, out=SC[:, 0:2], in0=SC[:, 2:4])
                assert snap == NSNAP, snap
                v.tensor_add(SC[:, 0:2], SC[:, 2:4], zbc2).then_inc(v_sem, 1)

            @block.gpsimd
            def _(g):
                g.wait_ge(v_sem, 1)
                g.dma_start(y_out[:], SN[:]).then_inc(dma_sem, 16)
                g.wait_ge(dma_sem, 32)

    mybir.codegen_inst_isa_subclasses(nc)
    return nc


def _interp_init(u0):
    """Replicate the reference's 1D border-padded linear interp, f32."""
    u0 = np.asarray(u0, dtype=np.float32)
    n_in = u0.shape[1]
    X = np.linspace(0.0, 1.0, MX, dtype=np.float32)
    pts = X * np.float32(2.0) - np.float32(1.0)
    idx = (pts + np.float32(1.0)) * np.float32(0.5) * np.float32(n_in - 1)
    idx = np.clip(idx, 0.0, np.float32(n_in - 1))
    i0 = np.floor(idx).astype(np.int32)
    i0 = np.clip(i0, 0, n_in - 2)
    frac = (idx - i0.astype(np.float32)).astype(np.float32)
    u0f = u0[:, i0] * (np.float32(1.0) - frac) + u0[:, i0 + 1] * frac
    return u0f[:, :-1].astype(np.float32)   # [B, 512]


def _in_maps(u0):
    """Per-core input tiles [128, TW]: dead cols + prefilled ghosts + pad."""
    u_init = _interp_init(u0)                       # [64, 512]
    w0 = (np.float32(C1) * u_init).astype(np.float32)
    cc, xx = np.meshgrid(np.arange(NCHUNK), np.arange(TW), indexing="ij")
    src = (cc * CH + xx - TB - H) % 512             # [16, TW]
    maps = []
    for core in range(NCORES):
        wrows = w0[core * BPC:(core + 1) * BPC]     # [8, 512]
        tile = wrows[:, src].astype(np.float32)     # [8, 16, TW]
        maps.append({"x": tile.reshape(128, TW)})
    return maps


def kernel(u0):
    from concourse.bass_utils import run_bass_kernel_spmd

    u0 = np.asarray(u0, dtype=np.float32)
    B = u0.shape[0]
    assert B == NCORES * BPC and u0.shape[1] == 512

    if "nc" not in _COMPILED:
        _COMPILED["nc"] = _build()
    nc = _COMPILED["nc"]

    res = run_bass_kernel_spmd(nc, _in_maps(u0), core_ids=list(range(NCORES)))

    out = np.empty((B, 257, NSNAP), dtype=np.float32)
    inv_c1 = np.float32(1.0 / C1)
    for core in range(NCORES):
        y = res.results[core]["y"]                  # [128, NSNAP*16]
        y = y.reshape(BPC, NCHUNK, NSNAP, 16)       # [b, chunk, t, k]
        u = y * inv_c1
        # spatial index nx = chunk*16 + k  (covers 0..255)
        out[core * BPC:(core + 1) * BPC, 0:256, :] = (
            u.transpose(0, 1, 3, 2).reshape(BPC, 256, NSNAP))
    out[:, 256, :] = out[:, 0, :]
    return out


# revision 20
# speedup vs baseline: 1.0324x; 1.0162x over previous
"""Trainium2 Bass kernel for nn_BurgersSolver_75333726371954.

Burgers' equation explicit solver: interpolate u0 [64,512] to a 513-point
grid, run 5000 sequential periodic-stencil steps on [64,512], snapshot every
50th step at every 2nd spatial point -> [64,257,101].

Strategy (pure data parallel, batch sharded 8 rows/core across 8 cores):
  * Scaled state w = C1*u so the update is
        w' = LIN*w + w_l*(C2+w) + w_r*(C2-w),   LIN = 1-2*C2
  * A custom DVE op (BURGERS_STEP2_ANT) computes one full time step per
    PASS over the stream, using two depth-1 temporal feedback taps
    (CURR_ALU_OUT z^-1 of the input stream for the left term, and z^-1 of
    an intermediate accumulator for combining the right term).  One input
    stream + one output stream; out base = in base - 1.
  * MULTI-PASS instructions: both APs get a stride-0 middle dim [K], so
    ONE instruction = K time steps (the ~75ns/instruction sequencer
    overhead and the ~12ns/pass row-restart are the only overheads beyond
    the ~1.05ns/element 1x DVE rate).  K <= 13 (hw row-count limit) and
    stream width >= ~104 elements (the src prefetcher runs ~103 elements
    ahead of compute; narrower passes read stale data -- PAD widens the
    stream to 110 for margin).
  * Layout [128 partitions = 8 batch x 16 spatial chunks of 32 sites,
    free = 2 dead cols + (H | 32 core | H) + 28 pad].  Ghost depth H=25
    allows 25 steps (K=12+13, two instructions) between halo refreshes;
    the valid region tapers by 1/side/step.
  * Halo refresh via 2 DVE stream_shuffle copies (partition rotation +-1
    within each 16-chunk group).  Everything runs on the vector engine in
    program order -- no PE, no PSUM, no cross-engine semaphores.
  * Snapshots every 50 steps land on block boundaries (50 = 2 x H); a
    custom 1x strided copy into an SBUF accumulator (doubling as the
    ghost-writeback spacer), single DMA out at the end.  Host rescales
    by 1/C1 and assembles the [64,257,101] output.
"""

import numpy as np

# ---- problem constants (hardcoded; must match the reference config) ----
MX = 513
MT = 5001
DX = 1.0 / (MX - 1)
DT = 1.0 / (MT - 1)
C1 = DT / (2.0 * DX)            # 0.0512
C2 = 0.005 * DT / DX ** 2       # 0.262144
LIN = float(1.0 - 2.0 * C2)

NSTEPS = MT - 1                 # 5000
SNAP_EVERY = 50
NSNAP = NSTEPS // SNAP_EVERY + 1  # 101

NCORES = 8
BPC = 8                         # batch rows per core
NCHUNK = 16                     # spatial chunks per batch row
CH = 32                         # chunk width (NCHUNK*CH == 512)
H = 25                          # ghost depth == steps per block
KSPLIT = (12, 13)               # passes per DVE instruction (sum == H; <= 13)
SW = CH + 2 * H                 # live state width (84)
PAD = 24                        # junk pad to keep the stream above the
                                # ~104-element src-prefetch staleness floor
WL = SW + PAD                   # instruction stream width (114)
TB = 2                          # dead leading cols (garbage landing zone)
TW = TB + WL                    # tile free width

_COMPILED = {}

# ---------------------------------------------------------------------------
# Custom DVE op: one Burgers step per pass, single input stream.
#
#   w'[c] = LIN*w[c] + w[c-1]*(C2+w[c]) + w[c+1]*(C2-w[c])
#
# Stream x = in0 (base b).  At position t (cur = x[b+t], xp = z^-1):
#   t_a = C2 + cur; t_b = C2 - xp; Q = xp*t_a; P = cur*t_b
#   A = LIN*cur + Q;  out[t] = z^-1(A) + P     -> center c = b+t-1
# out positions 0,1 are garbage (stale taps) and land on taper columns.
# ---------------------------------------------------------------------------

OP_NAME = "BURGERS_STEP2_ANT"


def _stencil_ref(in0, in1, s0, s1, imm2):
    out = np.zeros_like(in0)
    x = in0
    cur, xl, xr = x[:, 1:-1], x[:, :-2], x[:, 2:]
    out[:, 2:] = s0 * cur + xl * (s1 + cur) + xr * (s1 - cur)
    return out


def _build_uops():
    from concourse.dve_uop import (
        ENABLE, AluInp, AluOp, DelayInp, InpSel, OutPath, OutSel, Trigger,
        UopConfig,
    )

    u = UopConfig()
    u.enable_input(InpSel.SRC_0, 1)    # lane1 -> blk0's PREV_DELAY_0 port
    u.enable_input(InpSel.CONST_0, 2)  # LIN   -> PREV_DELAY_1
    u.enable_input(InpSel.CONST_1, 3)  # C2    -> PREV_DELAY_2
    u.require_inp0 = ENABLE
    u.trigger = (Trigger.SRC_TENSOR_DONE, Trigger.NONE, Trigger.NONE)
    u.next_uop = (0, 0, 0)
    u.enable_output(OutSel.ALU_OUT, OutPath.WR0_LO)
    dp = u.datapath_config
    # slice 0: stream -> flop0 (cur); d0 = z^-1 = xp; constants ride d1, d2
    dp[0].enable_alu(AluOp.BYPASS, AluInp.PREV_DELAY_0)
    dp[0].enable_delay_from_src(DelayInp.CURR_ALU_OUT, 0)    # d0 = xp
    dp[0].pass_through_delay(1, 2)
    # slice 1: t_a = cur + C2 ; keep cur in d3
    dp[1].enable_alu(AluOp.ADD, AluInp.PREV_ALU_OUT, AluInp.PREV_DELAY_2)
    dp[1].enable_delay_from_src(DelayInp.PREV_ALU_OUT, 3)    # d3 = cur
    dp[1].pass_through_delay(0, 1, 2)
    # slice 2: t_b = C2 - xp ; keep t_a in d4
    dp[2].enable_alu(AluOp.SUBTRACT, AluInp.PREV_DELAY_2, AluInp.PREV_DELAY_0)
    dp[2].enable_delay_from_src(DelayInp.PREV_ALU_OUT, 4)    # d4 = t_a
    dp[2].pass_through_delay(0, 1, 3)
    # slice 3: Q = xp * t_a ; keep t_b in d5
    dp[3].enable_alu(AluOp.MULTIPLY, AluInp.PREV_DELAY_0, AluInp.PREV_DELAY_4)
    dp[3].enable_delay_from_src(DelayInp.PREV_ALU_OUT, 5)    # d5 = t_b
    dp[3].pass_through_delay(1, 3)
    # slice 4: P = cur * t_b ; keep Q in d0
    dp[4].enable_alu(AluOp.MULTIPLY, AluInp.PREV_DELAY_3, AluInp.PREV_DELAY_5)
    dp[4].enable_delay_from_src(DelayInp.PREV_ALU_OUT, 0)    # d0 = Q
    dp[4].pass_through_delay(1, 3)
    # slice 5: A_lin = LIN * cur ; keep P in d1
    dp[5].enable_alu(AluOp.MULTIPLY, AluInp.PREV_DELAY_1, AluInp.PREV_DELAY_3)
    dp[5].enable_delay_from_src(DelayInp.PREV_ALU_OUT, 1)    # d1 = P
    dp[5].pass_through_delay(0)
    # slice 6: A = A_lin + Q ; d2 = z^-1(A) from own flop
    dp[6].enable_alu(AluOp.ADD, AluInp.PREV_ALU_OUT, AluInp.PREV_DELAY_0)
    dp[6].enable_delay_from_src(DelayInp.CURR_ALU_OUT, 2)    # d2 = A[t-1]
    dp[6].pass_through_delay(1)
    # slice 7: out = z^-1(A) + P
    dp[7].enable_alu(AluOp.ADD, AluInp.PREV_DELAY_2, AluInp.PREV_DELAY_1)
    u.validate("v3")
    return [u]


COPY_NAME = "COPY1X_ANT"


def _build_copy_uops():
    from concourse.dve_uop import (
        ENABLE, AluInp, AluOp, InpSel, OutPath, OutSel, Trigger, UopConfig,
    )

    u = UopConfig()
    u.enable_input(InpSel.SRC_0, 1)
    u.require_inp0 = ENABLE
    u.trigger = (Trigger.SRC_TENSOR_DONE, Trigger.NONE, Trigger.NONE)
    u.next_uop = (0, 0, 0)
    u.enable_output(OutSel.ALU_OUT, OutPath.WR0_LO)
    dp = u.datapath_config
    dp[0].enable_alu(AluOp.BYPASS, AluInp.PREV_DELAY_0)
    for i in range(1, 8):
        dp[i].enable_alu(AluOp.BYPASS, AluInp.PREV_ALU_OUT)
    u.validate("v3")
    return [u]


class _RawDveOp:
    """Duck-typed DveOp whose compile() returns hand-built uops."""

    def __init__(self, name, uop_builder, reference):
        from concourse.dve_spec import Spec, Src0

        self.name = name
        self.subdim = False
        self.perf_en = {}
        self.spec = Spec(body=Src0, reference=reference)
        self._uop_builder = uop_builder
        self._compiled = {}

    def compile(self, ver):
        if ver not in self._compiled:
            from concourse.dve_ops import get_dve_sub_opcode
            from concourse.dve_uop import DveOpSpec

            self._compiled[ver] = DveOpSpec(
                name=self.name,
                opcode=get_dve_sub_opcode(self.name),
                uops=self._uop_builder(),
                rd1_en=False,
            )
        return self._compiled[ver]


def _register_op(name, uop_builder, reference):
    import concourse.dve_ops as dve_ops

    for op in dve_ops.OPS:
        if op.name == name:
            return op
    op = _RawDveOp(name, uop_builder, reference)
    dve_ops.OPS.append(op)
    dve_ops.CUSTOM_DVE_SPECS[name] = op.spec
    dve_ops._SUB_OPCODE_FOR_NAME[name] = (
        max(dve_ops._SUB_OPCODE_FOR_NAME.values()) + 1
    )
    assert dve_ops._SUB_OPCODE_FOR_NAME[name] < 0x20
    return op


def _register_stencil_op():
    return _register_op(OP_NAME, _build_uops, _stencil_ref)


def _register_copy_op():
    return _register_op(COPY_NAME, _build_copy_uops,
                        lambda in0, in1, s0, s1, imm2: in0)


def _rot_mask(d):
    """out lane l takes input lane (l%16 + d)%16 within its 16-chunk group."""
    return [(l // 16) * 16 + ((l % 16) + d) % 16 for l in range(32)]


# ghost refresh pieces: (dst_lo, dst_hi, src_lo, rotation)  [state cols]
# left ghosts [0,H)  <- chunk c-1 cores [CH, CH+H) = state [2H-... wait:
#   ghost col g in [0,H): site offset g-H -> chunk c-1 col g-H+CH
#   = state col g-H+CH+H = g+CH
# right ghosts [H+CH, SW) <- chunk c+1 cols [0, H) = state cols [H, 2H)
_GHOSTS = [
    (0, H, CH, -1),               # left  [0,25)  <- c-1 state [32,57)
    (H + CH, SW, H, +1),          # right [57,84) <- c+1 state [25,50)
]


def _build():
    import concourse.bass as bass
    import concourse.mybir as mybir
    from concourse.ap import AP

    stencil = _register_stencil_op()
    copy1x = _register_copy_op()

    F32 = mybir.dt.float32

    nc = bass.Bass()
    x_in = nc.dram_tensor("x", [128, TW], F32, kind="ExternalInput")
    y_out = nc.dram_tensor("y", [128, NSNAP * 16], F32, kind="ExternalOutput")

    n_blocks = NSTEPS // H
    assert NSTEPS % H == 0 and sum(KSPLIT) == H and SNAP_EVERY % H == 0

    with (
        nc.semaphore("dma_sem") as dma_sem,
        nc.semaphore("v_sem") as v_sem,
        nc.sbuf_tensor("U", [128, TW], F32) as U,
        nc.sbuf_tensor("SN", [128, NSNAP * 16], F32) as SN,
        nc.sbuf_tensor("ZZ", [128, 1], F32) as ZZ,
        nc.sbuf_tensor("SC", [128, 4], F32) as SC,
    ):
        ub = U[:]
        ps = ub.ap[0][0]
        aps = {k: (AP(ub.tensor, TB, [[ps, 128], [0, k], [1, WL]]),
                   AP(ub.tensor, TB - 1, [[ps, 128], [0, k], [1, WL]]))
               for k in set(KSPLIT)}

        with nc.Block() as block:
            @block.gpsimd
            def _(g):
                g.memset(ZZ[:], 0.0)
                g.memset(SC[:], 0.0)
                g.dma_start(U[:], x_in[:]).then_inc(dma_sem, 16)

            zbc2 = ZZ[:].to_broadcast([128, 2])

            @block.vector
            def _(v):
                v.wait_ge(dma_sem, 16)

                def snapshot(k):
                    # custom 1x copy (a stock 2x copy could outrun writeback)
                    v._custom_dve(copy1x, out=SN[:, k * 16:k * 16 + 16],
                                  in0=U[:, TB + H:TB + H + CH:2])

                snapshot(0)
                snap = 1
                step = 0
                for blk in range(n_blocks):
                    for k in KSPLIT:
                        in0, out = aps[k]
                        v._custom_dve(stencil, out=out, in0=in0,
                                      s0=LIN, s1=C2)
                        step += k
                    if blk < n_blocks - 1:
                        for dlo, dhi, slo, rot in _GHOSTS:
                            w = dhi - dlo
                            v.stream_shuffle(U[:, TB + dlo:TB + dhi],
                                             U[:, TB + slo:TB + slo + w],
                                             _rot_mask(rot))
                    # snapshot doubles as the writeback-margin spacer between
                    # the ghost shuffles and the next step instruction's
                    # prefetch; on non-snapshot blocks use a dummy spacer
                    if step % SNAP_EVERY == 0:
                        snapshot(snap)
                        snap += 1
                    elif blk < n_blocks - 1:
                        v._custom_dve(copy1x, out=SC[:, 0:2], in0=SC[:, 2:4])
                assert snap == NSNAP, snap
                v.tensor_add(SC[:, 0:2], SC[:, 2:4], zbc2).then_inc(v_sem, 1)

            @block.gpsimd
            def _(g):
                g.wait_ge(v_sem, 1)
                g.dma_start(y_out[:], SN[:]).then_inc(dma_sem, 16)
                g.wait_ge(dma_sem, 32)

    mybir.codegen_inst_isa_subclasses(nc)
    return nc


def _interp_init(u0):
    """Replicate the reference's 1D border-padded linear interp, f32."""
    u0 = np.asarray(u0, dtype=np.float32)
    n_in = u0.shape[1]
    X = np.linspace(0.0, 1.0, MX, dtype=np.float32)
    pts = X * np.float32(2.0) - np.float32(1.0)
    idx = (pts + np.float32(1.0)) * np.float32(0.5) * np.float32(n_in - 1)
    idx = np.clip(idx, 0.0, np.float32(n_in - 1))
    i0 = np.floor(idx).astype(np.int32)
    i0 = np.clip(i0, 0, n_in - 2)
    frac = (idx - i0.astype(np.float32)).astype(np.float32)
    u0f = u0[:, i0] * (np.float32(1.0) - frac) + u0[:, i0 + 1] * frac
    return u0f[:, :-1].astype(np.float32)   # [B, 512]


def _in_maps(u0):
    """Per-core input tiles [128, TW]: dead cols + prefilled ghosts + pad."""
    u_init = _interp_init(u0)                       # [64, 512]
    w0 = (np.float32(C1) * u_init).astype(np.float32)
    cc, xx = np.meshgrid(np.arange(NCHUNK), np.arange(TW), indexing="ij")
    src = (cc * CH + xx - TB - H) % 512             # [16, TW]
    maps = []
    for core in range(NCORES):
        wrows = w0[core * BPC:(core + 1) * BPC]     # [8, 512]
        tile = wrows[:, src].astype(np.float32)     # [8, 16, TW]
        maps.append({"x": tile.reshape(128, TW)})
    return maps


def kernel(u0):
    from concourse.bass_utils import run_bass_kernel_spmd

    u0 = np.asarray(u0, dtype=np.float32)
    B = u0.shape[0]
    assert B == NCORES * BPC and u0.shape[1] == 512

    if "nc" not in _COMPILED:
        _COMPILED["nc"] = _build()
    nc = _COMPILED["nc"]

    res = run_bass_kernel_spmd(nc, _in_maps(u0), core_ids=list(range(NCORES)))

    out = np.empty((B, 257, NSNAP), dtype=np.float32)
    inv_c1 = np.float32(1.0 / C1)
    for core in range(NCORES):
        y = res.results[core]["y"]                  # [128, NSNAP*16]
        y = y.reshape(BPC, NCHUNK, NSNAP, 16)       # [b, chunk, t, k]
        u = y * inv_c1
        # spatial index nx = chunk*16 + k  (covers 0..255)
        out[core * BPC:(core + 1) * BPC, 0:256, :] = (
            u.transpose(0, 1, 3, 2).reshape(BPC, 256, NSNAP))
    out[:, 256, :] = out[:, 0, :]
    return out


# revision 21
# speedup vs baseline: 1.0326x; 1.0002x over previous
"""Trainium2 Bass kernel for nn_BurgersSolver_75333726371954.

Burgers' equation explicit solver: interpolate u0 [64,512] to a 513-point
grid, run 5000 sequential periodic-stencil steps on [64,512], snapshot every
50th step at every 2nd spatial point -> [64,257,101].

Strategy (pure data parallel, batch sharded 8 rows/core across 8 cores):
  * Scaled state w = C1*u so the update is
        w' = LIN*w + w_l*(C2+w) + w_r*(C2-w),   LIN = 1-2*C2
  * A custom DVE op (BURGERS_STEP2_ANT) computes one full time step per
    PASS over the stream, using two depth-1 temporal feedback taps
    (CURR_ALU_OUT z^-1 of the input stream for the left term, and z^-1 of
    an intermediate accumulator for combining the right term).  One input
    stream + one output stream; out base = in base - 1.
  * MULTI-PASS instructions: both APs get a stride-0 middle dim [K], so
    ONE instruction = K time steps (the ~75ns/instruction sequencer
    overhead and the ~12ns/pass row-restart are the only overheads beyond
    the ~1.05ns/element 1x DVE rate).  K <= 13 (hw row-count limit) and
    stream width >= ~104 elements (the src prefetcher runs ~103 elements
    ahead of compute; narrower passes read stale data -- PAD widens the
    stream to 106 for margin).
  * Layout [128 partitions = 8 batch x 16 spatial chunks of 32 sites,
    free = 2 dead cols + (H | 32 core | H) + 24 pad].  Ghost depth H=25
    allows 25 steps (K=12+13, two instructions) between halo refreshes;
    the valid region tapers by 1/side/step.
  * Halo refresh via 2 DVE stream_shuffle copies (partition rotation +-1
    within each 16-chunk group).  Everything runs on the vector engine in
    program order -- no PE, no PSUM, no cross-engine semaphores.
  * Snapshots every 50 steps land on block boundaries (50 = 2 x H); a
    custom 1x strided copy into an SBUF accumulator (doubling as the
    ghost-writeback spacer), single DMA out at the end.  Host rescales
    by 1/C1 and assembles the [64,257,101] output.
"""

import numpy as np

# ---- problem constants (hardcoded; must match the reference config) ----
MX = 513
MT = 5001
DX = 1.0 / (MX - 1)
DT = 1.0 / (MT - 1)
C1 = DT / (2.0 * DX)            # 0.0512
C2 = 0.005 * DT / DX ** 2       # 0.262144
LIN = float(1.0 - 2.0 * C2)

NSTEPS = MT - 1                 # 5000
SNAP_EVERY = 50
NSNAP = NSTEPS // SNAP_EVERY + 1  # 101

NCORES = 8
BPC = 8                         # batch rows per core
NCHUNK = 16                     # spatial chunks per batch row
CH = 32                         # chunk width (NCHUNK*CH == 512)
H = 25                          # ghost depth == steps per block
KSPLIT = (12, 13)               # passes per DVE instruction (sum == H; <= 13)
SW = CH + 2 * H                 # live state width (84)
PAD = 24                        # junk pad to keep the stream above the
                                # ~104-element src-prefetch staleness floor
WL = SW + PAD                   # instruction stream width (114)
TB = 2                          # dead leading cols (garbage landing zone)
TW = TB + WL                    # tile free width

_COMPILED = {}

# ---------------------------------------------------------------------------
# Custom DVE op: one Burgers step per pass, single input stream.
#
#   w'[c] = LIN*w[c] + w[c-1]*(C2+w[c]) + w[c+1]*(C2-w[c])
#
# Stream x = in0 (base b).  At position t (cur = x[b+t], xp = z^-1):
#   t_a = C2 + cur; t_b = C2 - xp; Q = xp*t_a; P = cur*t_b
#   A = LIN*cur + Q;  out[t] = z^-1(A) + P     -> center c = b+t-1
# out positions 0,1 are garbage (stale taps) and land on taper columns.
# ---------------------------------------------------------------------------

OP_NAME = "BURGERS_STEP2_ANT"


def _stencil_ref(in0, in1, s0, s1, imm2):
    out = np.zeros_like(in0)
    x = in0
    cur, xl, xr = x[:, 1:-1], x[:, :-2], x[:, 2:]
    out[:, 2:] = s0 * cur + xl * (s1 + cur) + xr * (s1 - cur)
    return out


def _build_uops():
    from concourse.dve_uop import (
        ENABLE, AluInp, AluOp, DelayInp, InpSel, OutPath, OutSel, Trigger,
        UopConfig,
    )

    u = UopConfig()
    u.enable_input(InpSel.SRC_0, 1)    # lane1 -> blk0's PREV_DELAY_0 port
    u.enable_input(InpSel.CONST_0, 2)  # LIN   -> PREV_DELAY_1
    u.enable_input(InpSel.CONST_1, 3)  # C2    -> PREV_DELAY_2
    u.require_inp0 = ENABLE
    u.trigger = (Trigger.SRC_TENSOR_DONE, Trigger.NONE, Trigger.NONE)
    u.next_uop = (0, 0, 0)
    u.enable_output(OutSel.ALU_OUT, OutPath.WR0_LO)
    dp = u.datapath_config
    # slice 0: stream -> flop0 (cur); d0 = z^-1 = xp; constants ride d1, d2
    dp[0].enable_alu(AluOp.BYPASS, AluInp.PREV_DELAY_0)
    dp[0].enable_delay_from_src(DelayInp.CURR_ALU_OUT, 0)    # d0 = xp
    dp[0].pass_through_delay(1, 2)
    # slice 1: t_a = cur + C2 ; keep cur in d3
    dp[1].enable_alu(AluOp.ADD, AluInp.PREV_ALU_OUT, AluInp.PREV_DELAY_2)
    dp[1].enable_delay_from_src(DelayInp.PREV_ALU_OUT, 3)    # d3 = cur
    dp[1].pass_through_delay(0, 1, 2)
    # slice 2: t_b = C2 - xp ; keep t_a in d4
    dp[2].enable_alu(AluOp.SUBTRACT, AluInp.PREV_DELAY_2, AluInp.PREV_DELAY_0)
    dp[2].enable_delay_from_src(DelayInp.PREV_ALU_OUT, 4)    # d4 = t_a
    dp[2].pass_through_delay(0, 1, 3)
    # slice 3: Q = xp * t_a ; keep t_b in d5
    dp[3].enable_alu(AluOp.MULTIPLY, AluInp.PREV_DELAY_0, AluInp.PREV_DELAY_4)
    dp[3].enable_delay_from_src(DelayInp.PREV_ALU_OUT, 5)    # d5 = t_b
    dp[3].pass_through_delay(1, 3)
    # slice 4: P = cur * t_b ; keep Q in d0
    dp[4].enable_alu(AluOp.MULTIPLY, AluInp.PREV_DELAY_3, AluInp.PREV_DELAY_5)
    dp[4].enable_delay_from_src(DelayInp.PREV_ALU_OUT, 0)    # d0 = Q
    dp[4].pass_through_delay(1, 3)
    # slice 5: A_lin = LIN * cur ; keep P in d1
    dp[5].enable_alu(AluOp.MULTIPLY, AluInp.PREV_DELAY_1, AluInp.PREV_DELAY_3)
    dp[5].enable_delay_from_src(DelayInp.PREV_ALU_OUT, 1)    # d1 = P
    dp[5].pass_through_delay(0)
    # slice 6: A = A_lin + Q ; d2 = z^-1(A) from own flop
    dp[6].enable_alu(AluOp.ADD, AluInp.PREV_ALU_OUT, AluInp.PREV_DELAY_0)
    dp[6].enable_delay_from_src(DelayInp.CURR_ALU_OUT, 2)    # d2 = A[t-1]
    dp[6].pass_through_delay(1)
    # slice 7: out = z^-1(A) + P
    dp[7].enable_alu(AluOp.ADD, AluInp.PREV_DELAY_2, AluInp.PREV_DELAY_1)
    u.validate("v3")
    return [u]


COPY_NAME = "COPY1X_ANT"


def _build_copy_uops():
    from concourse.dve_uop import (
        ENABLE, AluInp, AluOp, InpSel, OutPath, OutSel, Trigger, UopConfig,
    )

    u = UopConfig()
    u.enable_input(InpSel.SRC_0, 1)
    u.require_inp0 = ENABLE
    u.trigger = (Trigger.SRC_TENSOR_DONE, Trigger.NONE, Trigger.NONE)
    u.next_uop = (0, 0, 0)
    u.enable_output(OutSel.ALU_OUT, OutPath.WR0_LO)
    dp = u.datapath_config
    dp[0].enable_alu(AluOp.BYPASS, AluInp.PREV_DELAY_0)
    for i in range(1, 8):
        dp[i].enable_alu(AluOp.BYPASS, AluInp.PREV_ALU_OUT)
    u.validate("v3")
    return [u]


class _RawDveOp:
    """Duck-typed DveOp whose compile() returns hand-built uops."""

    def __init__(self, name, uop_builder, reference):
        from concourse.dve_spec import Spec, Src0

        self.name = name
        self.subdim = False
        self.perf_en = {}
        self.spec = Spec(body=Src0, reference=reference)
        self._uop_builder = uop_builder
        self._compiled = {}

    def compile(self, ver):
        if ver not in self._compiled:
            from concourse.dve_ops import get_dve_sub_opcode
            from concourse.dve_uop import DveOpSpec

            self._compiled[ver] = DveOpSpec(
                name=self.name,
                opcode=get_dve_sub_opcode(self.name),
                uops=self._uop_builder(),
                rd1_en=False,
            )
        return self._compiled[ver]


def _register_op(name, uop_builder, reference):
    import concourse.dve_ops as dve_ops

    for op in dve_ops.OPS:
        if op.name == name:
            return op
    op = _RawDveOp(name, uop_builder, reference)
    dve_ops.OPS.append(op)
    dve_ops.CUSTOM_DVE_SPECS[name] = op.spec
    dve_ops._SUB_OPCODE_FOR_NAME[name] = (
        max(dve_ops._SUB_OPCODE_FOR_NAME.values()) + 1
    )
    assert dve_ops._SUB_OPCODE_FOR_NAME[name] < 0x20
    return op


def _register_stencil_op():
    return _register_op(OP_NAME, _build_uops, _stencil_ref)


def _register_copy_op():
    return _register_op(COPY_NAME, _build_copy_uops,
                        lambda in0, in1, s0, s1, imm2: in0)


def _rot_mask(d):
    """out lane l takes input lane (l%16 + d)%16 within its 16-chunk group."""
    return [(l // 16) * 16 + ((l % 16) + d) % 16 for l in range(32)]


# ghost refresh pieces: (dst_lo, dst_hi, src_lo, rotation)  [state cols]
# left ghosts [0,H)  <- chunk c-1 cores [CH, CH+H) = state [2H-... wait:
#   ghost col g in [0,H): site offset g-H -> chunk c-1 col g-H+CH
#   = state col g-H+CH+H = g+CH
# right ghosts [H+CH, SW) <- chunk c+1 cols [0, H) = state cols [H, 2H)
_GHOSTS = [
    (0, H, CH, -1),               # left  [0,25)  <- c-1 state [32,57)
    (H + CH, SW, H, +1),          # right [57,84) <- c+1 state [25,50)
]


def _build():
    import concourse.bass as bass
    import concourse.mybir as mybir
    from concourse.ap import AP

    stencil = _register_stencil_op()
    copy1x = _register_copy_op()

    F32 = mybir.dt.float32

    nc = bass.Bass()
    x_in = nc.dram_tensor("x", [128, TW], F32, kind="ExternalInput")
    y_out = nc.dram_tensor("y", [128, NSNAP * 16], F32, kind="ExternalOutput")

    n_blocks = NSTEPS // H
    assert NSTEPS % H == 0 and sum(KSPLIT) == H and SNAP_EVERY % H == 0

    with (
        nc.semaphore("dma_sem") as dma_sem,
        nc.semaphore("v_sem") as v_sem,
        nc.sbuf_tensor("U", [128, TW], F32) as U,
        nc.sbuf_tensor("SN", [128, NSNAP * 16], F32) as SN,
        nc.sbuf_tensor("ZZ", [128, 1], F32) as ZZ,
        nc.sbuf_tensor("SC", [128, 4], F32) as SC,
    ):
        ub = U[:]
        ps = ub.ap[0][0]
        aps = {k: (AP(ub.tensor, TB, [[ps, 128], [0, k], [1, WL]]),
                   AP(ub.tensor, TB - 1, [[ps, 128], [0, k], [1, WL]]))
               for k in set(KSPLIT)}

        with nc.Block() as block:
            @block.gpsimd
            def _(g):
                g.memset(ZZ[:], 0.0)
                g.memset(SC[:], 0.0)
                g.dma_start(U[:], x_in[:]).then_inc(dma_sem, 16)

            zbc2 = ZZ[:].to_broadcast([128, 2])

            @block.vector
            def _(v):
                v.wait_ge(dma_sem, 16)

                def snapshot(k):
                    # custom 1x copy (a stock 2x copy could outrun writeback)
                    v._custom_dve(copy1x, out=SN[:, k * 16:k * 16 + 16],
                                  in0=U[:, TB + H:TB + H + CH:2])

                snapshot(0)
                snap = 1
                step = 0
                for blk in range(n_blocks):
                    for k in KSPLIT:
                        in0, out = aps[k]
                        v._custom_dve(stencil, out=out, in0=in0,
                                      s0=LIN, s1=C2)
                        step += k
                    if blk < n_blocks - 1:
                        for dlo, dhi, slo, rot in _GHOSTS:
                            w = dhi - dlo
                            v.stream_shuffle(U[:, TB + dlo:TB + dhi],
                                             U[:, TB + slo:TB + slo + w],
                                             _rot_mask(rot))
                    # snapshot doubles as the writeback-margin spacer between
                    # the ghost shuffles and the next step instruction's
                    # prefetch; on non-snapshot blocks use a dummy spacer
                    if step % SNAP_EVERY == 0:
                        snapshot(snap)
                        snap += 1
                    elif blk < n_blocks - 1:
                        v._custom_dve(copy1x, out=SC[:, 0:2], in0=SC[:, 2:4])
                assert snap == NSNAP, snap
                v.tensor_add(SC[:, 0:2], SC[:, 2:4], zbc2).then_inc(v_sem, 1)

            @block.gpsimd
            def _(g):
                g.wait_ge(v_sem, 1)
                g.dma_start(y_out[:], SN[:]).then_inc(dma_sem, 16)
                g.wait_ge(dma_sem, 32)

    mybir.codegen_inst_isa_subclasses(nc)
    return nc


def _interp_init(u0):
    """Replicate the reference's 1D border-padded linear interp, f32."""
    u0 = np.asarray(u0, dtype=np.float32)
    n_in = u0.shape[1]
    X = np.linspace(0.0, 1.0, MX, dtype=np.float32)
    pts = X * np.float32(2.0) - np.float32(1.0)
    idx = (pts + np.float32(1.0)) * np.float32(0.5) * np.float32(n_in - 1)
    idx = np.clip(idx, 0.0, np.float32(n_in - 1))
    i0 = np.floor(idx).astype(np.int32)
    i0 = np.clip(i0, 0, n_in - 2)
    frac = (idx - i0.astype(np.float32)).astype(np.float32)
    u0f = u0[:, i0] * (np.float32(1.0) - frac) + u0[:, i0 + 1] * frac
    return u0f[:, :-1].astype(np.float32)   # [B, 512]


def _in_maps(u0):
    """Per-core input tiles [128, TW]: dead cols + prefilled ghosts + pad."""
    u_init = _interp_init(u0)                       # [64, 512]
    w0 = (np.float32(C1) * u_init).astype(np.float32)
    cc, xx = np.meshgrid(np.arange(NCHUNK), np.arange(TW), indexing="ij")
    src = (cc * CH + xx - TB - H) % 512             # [16, TW]
    maps = []
    for core in range(NCORES):
        wrows = w0[core * BPC:(core + 1) * BPC]     # [8, 512]
        tile = wrows[:, src].astype(np.float32)     # [8, 16, TW]
        maps.append({"x": tile.reshape(128, TW)})
    return maps


def kernel(u0):
    from concourse.bass_utils import run_bass_kernel_spmd

    u0 = np.asarray(u0, dtype=np.float32)
    B = u0.shape[0]
    assert B == NCORES * BPC and u0.shape[1] == 512

    if "nc" not in _COMPILED:
        _COMPILED["nc"] = _build()
    nc = _COMPILED["nc"]

    res = run_bass_kernel_spmd(nc, _in_maps(u0), core_ids=list(range(NCORES)))

    out = np.empty((B, 257, NSNAP), dtype=np.float32)
    inv_c1 = np.float32(1.0 / C1)
    for core in range(NCORES):
        y = res.results[core]["y"]                  # [128, NSNAP*16]
        y = y.reshape(BPC, NCHUNK, NSNAP, 16)       # [b, chunk, t, k]
        u = y * inv_c1
        # spatial index nx = chunk*16 + k  (covers 0..255)
        out[core * BPC:(core + 1) * BPC, 0:256, :] = (
            u.transpose(0, 1, 3, 2).reshape(BPC, 256, NSNAP))
    out[:, 256, :] = out[:, 0, :]
    return out
